# revision 21
# baseline (speedup 1.0000x reference)
"""BeerLaNet proximal-gradient kernel for Trainium2, data-parallel over batch.

Contract: kernel(**inputs) takes FULL inputs (X [8,3,512,512], S [3,8],
gamma [1], lam [1], n_iter) and returns (x0 [8,3,1], S [3,8], Dt [8,8,512,512])
matching reference.py.

Design (per core = one sample):
  Layout A: SBUF [128, 16384] with partition (k*8+r), free j; p = k*16384+j.
  State Dt is factored as Dt[r,p] = f[r] * V[r,p] with V >= 0 stored on-chip
  (dtype BIGDT) and f tracked as an [8,1] per-row scalar.  All row reductions
  (sums / sums of squares) come out of a PE Gram matmul of W=[V;X;1] against
  itself, so no big reduction passes are needed.

  Per iteration:
    D-update:  PSUM = lhsT_V^T @ V + lhsT_X^T @ [X;1]  (PE, fp32r)
               V <- relu(PSUM)                          (ACT)
      where lhsT_V/lhsT_X fold f, tau_D, S^T X, S^T x0 and the L1 threshold
      so that relu(PSUM) == (prox result)/f exactly.
    Gram:      PE transposes of V tiles -> bf16 -> VV^T and XV^T accumulated
               in PSUM; masked+selector-reduced to M [8,8], Bt [8,3], vsum.
    S-update:  tiny [8,x] ops + one 64-float AllReduce across the 8 cores.
  The last iteration's S-update runs on the host (it only needs the tiny
  Gram stats), overlapping the 8MB V writeback with the device tail.

Measured (trn2, 8 cores, n_iter=3): relative L2 error vs the jax reference
  x0 4.3e-05 / S 8.6e-05 / Dt 3.7e-04 (fp32r state rounding dominates; the
  bf16 Gram contributes ~1e-5 thanks to 262k-term cancellation).
"""
import copy
import numpy as np

from concourse import bass, tile
from concourse import mybir
from concourse.bass_utils import run_bass_kernel_spmd

F32 = mybir.dt.float32
F32R = mybir.dt.float32r
BF16 = mybir.dt.bfloat16
AF = mybir.ActivationFunctionType
ALU = mybir.AluOpType

N, C, R, P1, P2 = 8, 3, 8, 512, 512
P = P1 * P2          # 262144
K = 16               # p-chunks per row
J = P // K           # 16384 free columns
W = 512              # D-sweep chunk width
NCH = J // W         # 32 chunks
NT = J // 128        # 128 gram tiles
EPS = 1e-10

BIGDT = F32R         # dtype of V / X / lhsT for the big matmuls
N_CORES = 8

# ---------------------------------------------------------------- wait fix
MAX_WAITS = 1


def _fix_multiwait(nc, max_waits=MAX_WAITS):
    """walrus here rejects >1 sem-wait per instruction; hoist extras onto
    NoOps inserted before the offender on the same (in-order) engine."""
    proto = nc.sync.nop().ins
    for f in nc.m.functions:
        for bb in f.blocks:
            lst = bb.instructions
            if lst and lst[-1] is proto:
                lst.pop()
    uid = 0
    for f in nc.m.functions:
        for bb in f.blocks:
            lst = bb.instructions
            out = []
            for inst in lst:
                si = getattr(inst, "sync_info", None)
                waits = list(si.on_wait) if si is not None else []
                if len(waits) > max_waits:
                    extras, keep = waits[:-max_waits], waits[-max_waits:]
                    for j in range(0, len(extras), max_waits):
                        nop = copy.deepcopy(proto)
                        nop.name = f"I-waitfix-{uid}"
                        uid += 1
                        nop.engine = inst.engine
                        nop.sync_info = mybir.SyncInfo(
                            on_wait=extras[j : j + max_waits], on_update=[]
                        )
                        out.append(nop)
                    inst.sync_info = mybir.SyncInfo(
                        on_wait=keep, on_update=list(si.on_update)
                    )
                out.append(inst)
            lst[:] = out


# ---------------------------------------------------------------- host consts
def host_consts():
    """Constant tensors shared by all cores."""
    c = {}
    c["ident_r"] = np.eye(128, dtype=np.float32)          # BIGDT identity
    c["ident32"] = np.eye(128, dtype=np.float32)
    # sel128[r', k*8+r] = (r == r')  : [8,1] -> [128,1] broadcast
    sel = np.zeros((8, 128), dtype=np.float32)
    for k in range(K):
        for r in range(8):
            sel[r, k * 8 + r] = 1.0
    c["sel128"] = sel
    # selK[k*8+r, r'] = (r == r')   : k-sum selector
    c["selK"] = sel.T.copy()
    # sel48[c', k*3+c] = (c == c')
    s48 = np.zeros((3, 64), dtype=np.float32)
    for k in range(K):
        for cc in range(3):
            s48[cc, k * 3 + cc] = 1.0
    c["sel48"] = s48
    c["ones18"] = np.ones((1, 8), dtype=np.float32)
    c["ones13"] = np.ones((1, 3), dtype=np.float32)
    c["ones31"] = np.ones((3, 1), dtype=np.float32)
    c["ones81"] = np.ones((8, 1), dtype=np.float32)
    c["i8"] = np.eye(8, dtype=np.float32)
    # block-diag masks
    bd = np.zeros((128, 128), dtype=np.float32)
    for k in range(K):
        bd[k * 8 : k * 8 + 8, k * 8 : k * 8 + 8] = 1.0
    c["bd128"] = bd
    b48 = np.zeros((64, 128), dtype=np.float32)
    for k in range(K):
        b48[k * 3 : k * 3 + 3, k * 8 : k * 8 + 8] = 1.0
    c["bd48"] = b48
    # gram mask [128, 177]
    gm = np.zeros((128, 177), dtype=np.float32)
    for k in range(K):
        gm[k * 8 : k * 8 + 8, k * 8 : k * 8 + 8] = 1.0          # VV^T diag blocks
        gm[k * 8 : k * 8 + 8, 128 + k * 3 : 128 + k * 3 + 3] = 1.0  # XV^T
    gm[:, 176] = 1.0                                            # ones column
    c["gmask"] = gm
    return c


def host_iter0(S, gamma, lam, meanX_core):
    """Host-computed stage-A for iteration 0 (f=1, vsum=0, x0=meanX)."""
    S = S.astype(np.float64)
    A = S.T @ S
    tau_D = 1.0 / A.trace()
    snrm = np.sqrt(np.diag(A))
    thr = lam * gamma * tau_D * snrm                     # [8]
    # lhsT_X rows (k,c): -tau_D * S[c,r]  (f=1)
    lx = np.zeros((65, 128), dtype=np.float32)
    for k in range(K):
        for cc in range(3):
            for r in range(8):
                lx[k * 3 + cc, k * 8 + r] = -tau_D * S[cc, r]
    # c-row: tau_D * (S^T x0)[r] - thr[r] with x0 = meanX
    stx0 = S.T @ meanX_core.reshape(3)
    c2 = tau_D * stx0 - thr
    for k in range(K):
        lx[64, k * 8 : k * 8 + 8] = c2
    return lx.astype(np.float32), tau_D, snrm


# ---------------------------------------------------------------- builder
def build_nc(n_iter: int):
    nc = bass.Bass()
    dram = {}

    def din(name, shape, dt=F32):
        dram[name] = nc.dram_tensor(name, shape, dt, kind="ExternalInput")
        return dram[name]

    def dout(name, shape, dt=F32):
        dram[name] = nc.dram_tensor(name, shape, dt, kind="ExternalOutput")
        return dram[name]

    X_in = din("Xin", [65, J], BIGDT)
    XT_in = din("XTin", [128, NT * 49], BF16)
    lx1_in = din("lx1", [65, 128], BIGDT)
    St_in = din("St0", [8, 3])
    meanX_in = din("meanX", [3, 1])
    f0_in = din("f0", [8, 1])
    tauD0_in = din("tauD0", [8, 1])
    snrm0_in = din("snrm0", [8, 1])
    l8_in = din("l8", [8, 1])        # lam
    lg8_in = din("lg8", [8, 1])      # lam*gamma
    g8_in = din("g8", [8, 1])        # gamma

    consts = host_consts()
    cin = {}
    for name, arr in consts.items():
        dt = BIGDT if name == "ident_r" else (F32)
        cin[name] = din(name, list(arr.shape), dt)

    V_out = dout("V_out", [128, J])
    gram_raw_out = dout("gram_raw", [128, 177])
    fs_out = dout("fs_out", [8, 8])

    cc_groups = [list(range(N_CORES))]

    with tile.TileContext(nc) as tc:
        with tc.tile_pool(name="big", bufs=1) as bigpool, \
             tc.tile_pool(name="stage", bufs=3) as stpool, \
             tc.tile_pool(name="small", bufs=2) as sm, \
             tc.tile_pool(name="psweep", bufs=2, space="PSUM") as psweep, \
             tc.tile_pool(name="pgram", bufs=1, space="PSUM") as pgram, \
             tc.tile_pool(name="ptp", bufs=2, space="PSUM") as ptp, \
             tc.tile_pool(name="psmall", bufs=2, space="PSUM") as psm, \
             tc.tile_pool(name="dram", bufs=2, space="DRAM") as dpool:

            # ---------- persistent SBUF state ----------
            V = bigpool.tile([128, J], BIGDT, tag="V", name="V")
            X49 = bigpool.tile([65, J], BIGDT, tag="X49", name="X49")
            XT16 = bigpool.tile([128, NT * 49], BF16, tag="XT16", name="XT16")
            nc.sync.dma_start(out=X49[:], in_=X_in[:])
            nc.sync.dma_start(out=XT16[:], in_=XT_in[:])

            cst = {}
            for name, arr in consts.items():
                dt = BIGDT if name == "ident_r" else F32
                cst[name] = bigpool.tile(list(arr.shape), dt, tag=f"c_{name}", name=f"c_{name}")
                nc.sync.dma_start(out=cst[name][:], in_=cin[name][:])

            lxt1 = bigpool.tile([65, 128], BIGDT, tag="lx1", name="lx1t")
            nc.sync.dma_start(out=lxt1[:], in_=lx1_in[:])
            St = bigpool.tile([8, 3], F32, tag="St", name="Stt")
            nc.sync.dma_start(out=St[:], in_=St_in[:])
            meanX = bigpool.tile([3, 1], F32, tag="meanX", name="meanXt")
            nc.sync.dma_start(out=meanX[:], in_=meanX_in[:])
            smallins = {}
            for name, t_in in (("f0", f0_in), ("tauD0", tauD0_in),
                               ("snrm0", snrm0_in), ("l8", l8_in),
                               ("lg8", lg8_in), ("g8", g8_in)):
                smallins[name] = bigpool.tile([8, 1], F32, tag=f"s_{name}", name=f"s_{name}")
                nc.sync.dma_start(out=smallins[name][:], in_=t_in[:])

            ident_r = cst["ident_r"]
            i8 = cst["i8"]

            # helpers --------------------------------------------------
            def ts(out, in0, s1, op0, s2=None, op1=None):
                if op1 is None:
                    nc.vector.tensor_scalar(out, in0, s1, None, op0)
                else:
                    nc.vector.tensor_scalar(out, in0, s1, s2, op0, op1)

            def tt(out, a, b, op):
                nc.vector.tensor_tensor(out=out, in0=a, in1=b, op=op)

            def small(shape, tag, dt=F32):
                return sm.tile(shape, dt, tag=tag, name=tag)

            def psmall(shape, tag):
                return psm.tile(shape, F32, tag="ps_small", name=tag)

            def pe_bcast(rhs_ap, npart, tag):
                """broadcast a [1, m] row (or [1,1]) to [npart, m] via PE."""
                lhs = cst["ones18"][:, 0:npart] if npart <= 8 else None
                assert lhs is not None
                m = rhs_ap.shape[-1]
                ps = psmall([npart, m], tag)
                nc.tensor.matmul(ps[:], lhs, rhs_ap, start=True, stop=True)
                return ps

            def transpose_small(in_ap, rows, cols, tag):
                """[rows, cols] -> [cols, rows] via PE (fp32)."""
                ps = psm.tile([cols, rows], F32, tag="ps_small", name=tag)
                nc.tensor.transpose(ps[:], in_ap, cst["ident32"][0:rows, 0:rows])
                out = small([cols, rows], tag + "_sb")
                nc.vector.tensor_copy(out[:], ps[:])
                return out

            # ---------- per-iteration python-side state ----------
            f8 = smallins["f0"]          # [8,1] current f
            tauD8 = smallins["tauD0"]    # [8,1]
            snrm8 = smallins["snrm0"]    # [8,1]
            lxt = lxt1                   # current lhsT_X [49,128] BIGDT
            lvt = None                   # current lhsT_V [128,128] BIGDT
            l8 = smallins["l8"]
            lg8 = smallins["lg8"]
            g8 = smallins["g8"]

            for it in range(n_iter):
                last = it == n_iter - 1
                # ================= D-update sweep =================
                for ch in range(NCH):
                    sl = slice(ch * W, (ch + 1) * W)
                    ps = psweep.tile([128, W], F32, tag="sweep", name="sweep")
                    if it > 0:
                        nc.tensor.matmul(ps[:], lvt[:], V[:, sl], start=True,
                                         stop=False)
                    nc.tensor.matmul(ps[:], lxt[:], X49[:, sl], start=(it == 0),
                                     stop=True)
                    nc.scalar.activation(V[:, sl], ps[:], AF.Relu)
                    if last:
                        # stream V out as soon as each chunk is final
                        nc.sync.dma_start(out=V_out[:, sl],
                                          in_=V[:, sl].bitcast(F32))

                # ================= Gram =================
                gpsA = pgram.tile([128, 128], F32, tag="gramA", name="gramA")
                gpsB = pgram.tile([128, 49], F32, tag="gramB", name="gramB")
                for g in range(NT // 4):
                    tp = ptp.tile([128, 512], BIGDT, tag="tp", name="tp")
                    for u in range(4):
                        t = g * 4 + u
                        nc.tensor.transpose(tp[:, u * 128 : u * 128 + 128],
                                            V[:, t * 128 : t * 128 + 128],
                                            ident_r[:])
                    st = stpool.tile([128, 512], BF16, tag="vt16", name="vt16")
                    nc.vector.tensor_copy(st[:], tp[:].bitcast(F32))
                    for u in range(4):
                        t = g * 4 + u
                        usl = slice(u * 128, u * 128 + 128)
                        nc.tensor.matmul(gpsA[:], st[:, usl], st[:, usl],
                                         start=(t == 0), stop=(t == NT - 1))
                        nc.tensor.matmul(gpsB[:], st[:, usl],
                                         XT16[:, t * 49 : t * 49 + 49],
                                         start=(t == 0), stop=(t == NT - 1))

                if last:
                    go = small([128, 177], "gramraw")
                    nc.vector.tensor_copy(go[:, 0:128], gpsA[:])
                    nc.vector.tensor_copy(go[:, 128:177], gpsB[:])
                    nc.sync.dma_start(out=gram_raw_out[:], in_=go[:])
                    fsb = small([8, 8], "fsb")
                    nc.vector.memset(fsb[:], 0.0)
                    nc.vector.tensor_copy(fsb[:, 0:1], f8[:])
                    nc.vector.tensor_copy(fsb[:, 1:4], St[:])
                    nc.sync.dma_start(out=fs_out[:], in_=fsb[:])
                    break

                # ---------- Gram reduce: M, Bt, vsum, vsq ----------
                gsb = small([128, 177], "gsb")
                tt(gsb[:, 0:128], gpsA[:], cst["gmask"][:, 0:128], ALU.mult)
                tt(gsb[:, 128:177], gpsB[:], cst["gmask"][:, 128:177], ALU.mult)
                kgp = psm.tile([8, 177], F32, tag="ps_small", name="kgp")
                nc.tensor.matmul(kgp[:], cst["selK"][:], gsb[:], start=True,
                                 stop=True)
                KG = small([8, 177], "KG")
                nc.vector.tensor_copy(KG[:], kgp[:])
                M = small([8, 8], "M")
                nc.vector.tensor_reduce(
                    M[:], KG[:, 0:128].rearrange("p (k r) -> p r k", k=K, r=8),
                    mybir.AxisListType.X, ALU.add)
                Bt = small([8, 3], "Bt")
                nc.vector.tensor_reduce(
                    Bt[:], KG[:, 128:176].rearrange("p (k c) -> p c k", k=K, c=3),
                    mybir.AxisListType.X, ALU.add)
                vsum = small([8, 1], "vsum")
                nc.vector.tensor_copy(vsum[:], KG[:, 176:177])
                vsq = small([8, 1], "vsq")
                scr8 = small([8, 8], "scr8")
                tt(scr8[:], M[:], i8[:], ALU.mult)
                nc.vector.tensor_reduce(vsq[:], scr8[:], mybir.AxisListType.X,
                                        ALU.add)

                # ---------- local D-prox scalars ----------
                sqv = small([8, 1], "sqv")
                nc.scalar.activation(sqv[:], vsq[:], AF.Sqrt)
                DtL2 = small([8, 1], "DtL2")
                tt(DtL2[:], f8[:], sqv[:], ALU.mult)
                thrB = small([8, 1], "thrB")
                ts(thrB[:], snrm8[:], l8[:], ALU.mult)
                ts(thrB[:], thrB[:], tauD8[:], ALU.mult)
                t3 = small([8, 1], "t3")
                tt(t3[:], DtL2[:], thrB[:], ALU.subtract)
                ts(t3[:], t3[:], 0.0, ALU.max)
                recL2 = small([8, 1], "recL2")
                nc.vector.reciprocal(recL2[:], DtL2[:])
                scl = small([8, 1], "scl")
                tt(scl[:], t3[:], recL2[:], ALU.mult)
                ts(scl[:], scl[:], EPS, ALU.add)
                f1 = small([8, 1], "f1")
                tt(f1[:], f8[:], scl[:], ALU.mult)

                # ---------- pre-reduce: G^T etc ----------
                fvs = small([8, 1], "fvs")
                tt(fvs[:], f1[:], vsum[:], ALU.mult)
                x0p = psmall([3, 1], "x0p")
                nc.tensor.matmul(x0p[:], St[:], fvs[:], start=True, stop=True)
                x0 = small([3, 1], "x0")
                nc.vector.tensor_scalar(x0[:], x0p[:], 1.0 / P, None, ALU.mult)
                tt(x0[:], x0[:], meanX[:], ALU.add)

                f1row = transpose_small(f1[:], 8, 1, "f1row")      # [1,8]
                fbc = pe_bcast(f1row[:], 8, "fbc")                 # [8,8] psum
                Mf = small([8, 8], "Mf")
                ts(Mf[:], M[:], f1[:], ALU.mult)
                tt(Mf[:], Mf[:], fbc[:], ALU.mult)                 # f_i f_j M
                Btf = small([8, 3], "Btf")
                ts(Btf[:], Bt[:], f1[:], ALU.mult)

                gt_ps = psmall([8, 3], "gt")
                nc.tensor.matmul(gt_ps[:], Mf[:], St[:], start=True, stop=False)
                nc.tensor.matmul(gt_ps[:], i8[:], Btf[:], start=False, stop=False)
                nfvs = small([8, 1], "nfvs")
                ts(nfvs[:], fvs[:], -1.0, ALU.mult)
                nfvs_row = transpose_small(nfvs[:], 8, 1, "nfvsrow")  # [1,8]
                x0row = transpose_small(x0[:], 3, 1, "x0row")         # [1,3]
                nc.tensor.matmul(gt_ps[:], nfvs_row[:], x0row[:], start=False,
                                 stop=True)
                Gt = small([8, 3], "Gt")
                nc.vector.tensor_copy(Gt[:], gt_ps[:])

                dtn = small([8, 1], "dtn")
                tt(dtn[:], g8[:], fvs[:], ALU.mult)
                tb = small([8, 1], "tb_")
                tt(tb[:], f1[:], sqv[:], ALU.mult)
                tt(dtn[:], dtn[:], tb[:], ALU.add)
                fro = small([8, 1], "fro")
                tt(fro[:], f1[:], f1[:], ALU.mult)
                tt(fro[:], fro[:], vsq[:], ALU.mult)

                alr = small([8, 8], "alr")
                nc.vector.memset(alr[:], 0.0)
                nc.vector.tensor_copy(alr[:, 0:3], Gt[:])
                nc.vector.tensor_copy(alr[:, 3:4], dtn[:])
                nc.vector.tensor_copy(alr[:, 4:5], fro[:])

                cc_in = dpool.tile([8, 8], F32, name="cc_in")
                cc_out = dpool.tile([8, 8], F32, addr_space="Shared", name="cc_out")
                nc.sync.dma_start(out=cc_in[:], in_=alr[:])
                nc.gpsimd.collective_compute(
                    "AllReduce", ALU.add, replica_groups=cc_groups,
                    ins=[cc_in.opt()], outs=[cc_out.opt()])
                Rr = small([8, 8], "Rr")
                nc.sync.dma_start(out=Rr[:], in_=cc_out[:])

                # ---------- stage B: S update ----------
                frosum_ps = psmall([1, 1], "frosum")
                nc.tensor.matmul(frosum_ps[:], cst["ones81"][:], Rr[:, 4:5],
                                 start=True, stop=True)
                tauS = small([1, 1], "tauS")
                nc.vector.reciprocal(tauS[:], frosum_ps[:])
                ts(tauS[:], tauS[:], float(N_CORES), ALU.mult)
                tauS8 = psm.tile([8, 1], F32, tag="ps_small", name="tauS8")
                nc.tensor.matmul(tauS8[:], cst["ones18"][:], tauS[:],
                                 start=True, stop=True)
                tauS8s = small([8, 1], "tauS8s")
                nc.vector.tensor_copy(tauS8s[:], tauS8[:])

                Sg = small([8, 3], "Sg")
                ts(Sg[:], Rr[:, 0:3], tauS8s[:], ALU.mult, 1.0 / N_CORES,
                   ALU.mult)
                tt(Sg[:], St[:], Sg[:], ALU.subtract)
                scrg = small([8, 3], "scrg")
                sg2 = small([8, 1], "sg2")
                tt(scrg[:], Sg[:], Sg[:], ALU.mult)
                nc.vector.tensor_reduce(sg2[:], scrg[:], mybir.AxisListType.X,
                                        ALU.add)
                snrm2 = small([8, 1], "snrm2")
                nc.scalar.activation(snrm2[:], sg2[:], AF.Sqrt)
                dtnm = small([8, 1], "dtnm")
                ts(dtnm[:], Rr[:, 3:4], 1.0 / N_CORES, ALU.mult)
                t1 = small([8, 1], "sb_t1")
                tt(t1[:], dtnm[:], tauS8s[:], ALU.mult)
                tt(t1[:], t1[:], l8[:], ALU.mult)
                t2 = small([8, 1], "sb_t2")
                tt(t2[:], snrm2[:], t1[:], ALU.subtract)
                ts(t2[:], t2[:], 0.0, ALU.max)
                t5 = small([8, 1], "sb_t5")
                ts(t5[:], snrm2[:], EPS, ALU.add)
                rec5 = small([8, 1], "sb_rec5")
                nc.vector.reciprocal(rec5[:], t5[:])
                sclS = small([8, 1], "sclS")
                tt(sclS[:], t2[:], rec5[:], ALU.mult)
                Sn = small([8, 3], "Sn")
                ts(Sn[:], Sg[:], sclS[:], ALU.mult)
                ns_scr = small([8, 3], "ns_scr")
                ns2 = small([8, 1], "ns2")
                tt(ns_scr[:], Sn[:], Sn[:], ALU.mult)
                nc.vector.tensor_reduce(ns2[:], ns_scr[:], mybir.AxisListType.X,
                                        ALU.add)
                nrm3 = small([8, 1], "nrm3")
                nc.scalar.activation(nrm3[:], ns2[:], AF.Sqrt)
                nrm3e = small([8, 1], "nrm3e")
                ts(nrm3e[:], nrm3[:], EPS, ALU.add)
                recn3 = small([8, 1], "recn3")
                nc.vector.reciprocal(recn3[:], nrm3e[:])
                St_new = small([8, 3], "St_new")
                ts(St_new[:], Sn[:], recn3[:], ALU.mult)
                f_new = small([8, 1], "f_new")
                tt(f_new[:], f1[:], nrm3e[:], ALU.mult)

                # ---------- stage A for next iteration ----------
                Ssb_ps = psm.tile([3, 8], F32, tag="ps_small", name="Ssb_ps")
                nc.tensor.transpose(Ssb_ps[:], St_new[:],
                                    cst["ident32"][0:8, 0:8])
                Ssb = small([3, 8], "Ssb")
                nc.vector.tensor_copy(Ssb[:], Ssb_ps[:])

                A_ps = psmall([8, 8], "A_ps")
                nc.tensor.matmul(A_ps[:], Ssb[:], Ssb[:], start=True, stop=True)
                Amat = small([8, 8], "Amat")
                nc.vector.tensor_copy(Amat[:], A_ps[:])
                Ssq = small([3, 8], "Ssq")
                nc.scalar.activation(Ssq[:], Ssb[:], AF.Square)
                n2_ps = psmall([1, 8], "n2_ps")
                nc.tensor.matmul(n2_ps[:], cst["ones31"][:], Ssq[:], start=True,
                                 stop=True)
                n2row = small([1, 8], "n2row")
                nc.vector.tensor_copy(n2row[:], n2_ps[:])
                tot = small([1, 1], "tot")
                nc.vector.tensor_reduce(tot[:], n2row[:], mybir.AxisListType.X,
                                        ALU.add)
                tauD = small([1, 1], "tauD")
                nc.vector.reciprocal(tauD[:], tot[:])
                snrmrow = small([1, 8], "snrmrow")
                nc.scalar.activation(snrmrow[:], n2row[:], AF.Sqrt)
                snrm8_n = transpose_small(snrmrow[:], 1, 8, "snrm8n")  # [8,1]
                tauD8n_ps = pe_bcast(tauD[:], 8, "tauD8n")
                tauD8n = small([8, 1], "tauD8nsb")
                nc.vector.tensor_copy(tauD8n[:], tauD8n_ps[:])
                tauD3_ps = pe_bcast(tauD[:], 3, "tauD3")
                tauD3 = small([3, 1], "tauD3sb")
                nc.vector.tensor_copy(tauD3[:], tauD3_ps[:])

                recf = small([8, 1], "recf")
                nc.vector.reciprocal(recf[:], f_new[:])
                recfrow = transpose_small(recf[:], 8, 1, "recfrow")   # [1,8]

                # B = I - tauD * f_i A_ij / f_j
                bt1 = small([8, 8], "bt1")
                ts(bt1[:], Amat[:], f_new[:], ALU.mult)
                recbc = pe_bcast(recfrow[:], 8, "recbc")              # [8,8]
                tt(bt1[:], bt1[:], recbc[:], ALU.mult)
                ts(bt1[:], bt1[:], tauD8n[:], ALU.mult)
                Bm = small([8, 8], "Bm")
                tt(Bm[:], i8[:], bt1[:], ALU.subtract)
                bcB_ps = psm.tile([128, 8], F32, tag="ps_small", name="bcB_ps")
                nc.tensor.matmul(bcB_ps[:], cst["sel128"][:], Bm[:], start=True,
                                 stop=True)
                bcB = small([128, 8], "bcBsb")
                nc.vector.tensor_copy(bcB[:], bcB_ps[:])
                lvt_f = stpool.tile([128, 128], F32, tag="lvtf", name="lvtf")
                tt(lvt_f[:].rearrange("p (k r) -> p k r", k=K, r=8),
                   bcB[:].unsqueeze(1).broadcast_to([128, K, 8]),
                   cst["bd128"][:].rearrange("p (k r) -> p k r", k=K, r=8),
                   ALU.mult)
                lvt_new = stpool.tile([128, 128], BIGDT, tag="lvt", name="lvt")
                ts(lvt_new[:], lvt_f[:], 1.0, ALU.mult)

                # lhsT_X rows: -tauD * S[c,r] / f[r]
                xb1 = small([3, 8], "xb1")
                rec3bc = pe_bcast(recfrow[:], 3, "rec3bc")            # [3,8]
                tt(xb1[:], Ssb[:], rec3bc[:], ALU.mult)
                ts(xb1[:], xb1[:], tauD3[:], ALU.mult)
                ts(xb1[:], xb1[:], -1.0, ALU.mult)
                bc48_ps = psm.tile([64, 8], F32, tag="ps_small", name="bc48_ps")
                nc.tensor.matmul(bc48_ps[:], cst["sel48"][:], xb1[:],
                                 start=True, stop=True)
                bc48 = small([64, 8], "bc48sb")
                nc.vector.tensor_copy(bc48[:], bc48_ps[:])
                lxt_f = stpool.tile([65, 128], F32, tag="lxtf", name="lxtf")
                tt(lxt_f[0:64, :].rearrange("p (k r) -> p k r", k=K, r=8),
                   bc48[:].unsqueeze(1).broadcast_to([64, K, 8]),
                   cst["bd48"][:].rearrange("p (k r) -> p k r", k=K, r=8),
                   ALU.mult)

                # c-row: (tauD * (S^T x0') - thr) / f ; x0' uses f_new,vsum
                fvs2 = small([8, 1], "fvs2")
                tt(fvs2[:], f_new[:], vsum[:], ALU.mult)
                x0n_ps = psmall([3, 1], "x0n_ps")
                nc.tensor.matmul(x0n_ps[:], St_new[:], fvs2[:], start=True,
                                 stop=True)
                x0n = small([3, 1], "x0n")
                nc.vector.tensor_scalar(x0n[:], x0n_ps[:], 1.0 / P, None,
                                        ALU.mult)
                tt(x0n[:], x0n[:], meanX[:], ALU.add)
                stx0_ps = psmall([8, 1], "stx0")
                nc.tensor.matmul(stx0_ps[:], Ssb[:], x0n[:], start=True,
                                 stop=True)
                c2 = small([8, 1], "c2")
                nc.vector.tensor_scalar(c2[:], stx0_ps[:], tauD8n[:], None,
                                        ALU.mult)
                thrN = small([8, 1], "thrN")
                tt(thrN[:], snrm8_n[:], lg8[:], ALU.mult)
                tt(thrN[:], thrN[:], tauD8n[:], ALU.mult)
                tt(c2[:], c2[:], thrN[:], ALU.subtract)
                tt(c2[:], c2[:], recf[:], ALU.mult)
                c2row = transpose_small(c2[:], 8, 1, "c2row")         # [1,8]
                nc.vector.tensor_copy(
                    lxt_f[64:65, :].rearrange("p (k r) -> p k r", k=K, r=8),
                    c2row[:].unsqueeze(1).broadcast_to([1, K, 8]))
                lxt_new = stpool.tile([65, 128], BIGDT, tag="lxt", name="lxt")
                ts(lxt_new[:], lxt_f[:], 1.0, ALU.mult)

                # rotate python-side state
                f8, tauD8, snrm8 = f_new, tauD8n, snrm8_n
                lvt, lxt, St = lvt_new, lxt_new, St_new

    return nc


# ---------------------------------------------------------------- host side
def _prep_core_inputs(Xi, S, gamma, lam, consts):
    """Xi: [3, P] float32 for this core."""
    out = {}
    Xk = Xi.reshape(3, K, J)                       # [c,k,j]
    x49 = np.zeros((65, J), dtype=np.float32)
    for k in range(K):
        x49[k * 3 : k * 3 + 3, :] = Xk[:, k, :]
    x49[64, :] = 1.0
    out["Xin"] = x49
    # XT16[q, t*49 + k*3+c] = X[c, k*16384 + t*128 + q]; col 48 of block = 1
    Xt = Xi.reshape(3, K, NT, 128)                 # [c,k,t,q]
    xt = np.ones((128, NT, 49), dtype=np.float32)
    # -> [q, t, k*3+c]
    xt[:, :, 0:48] = np.transpose(Xt, (3, 2, 1, 0)).reshape(128, NT, 48)
    out["XTin"] = xt.reshape(128, NT * 49).astype(mybir.dt.np(BF16))
    out["meanX"] = Xi.mean(axis=1, dtype=np.float64).astype(np.float32).reshape(3, 1)
    lx1, tauD, snrm = host_iter0(S, float(gamma), float(lam), out["meanX"])
    out["lx1"] = lx1
    out["St0"] = S.T.astype(np.float32).copy()
    out["f0"] = np.ones((8, 1), np.float32)
    out["tauD0"] = np.full((8, 1), tauD, np.float32)
    out["snrm0"] = snrm.reshape(8, 1).astype(np.float32)
    out["l8"] = np.full((8, 1), float(lam), np.float32)
    out["lg8"] = np.full((8, 1), float(lam) * float(gamma), np.float32)
    out["g8"] = np.full((8, 1), float(gamma), np.float32)
    for name, arr in consts.items():
        out[name] = arr
    return out


RUNNER = None  # test hook: (nc, in_maps) -> results
_NC_CACHE = {}


def kernel(X, S, gamma, lam, n_iter):
    n_iter = int(n_iter)
    X = np.asarray(X, dtype=np.float32)
    S = np.asarray(S, dtype=np.float32)
    gamma_a = abs(float(np.asarray(gamma).reshape(-1)[0]))
    lam_a = abs(float(np.asarray(lam).reshape(-1)[0]))

    if n_iter not in _NC_CACHE:
        _NC_CACHE[n_iter] = build_nc(n_iter)
    nc = _NC_CACHE[n_iter]

    consts = host_consts()
    Xf = X.reshape(N, C, P)
    in_maps = [
        _prep_core_inputs(Xf[i], S, gamma_a, lam_a, consts) for i in range(N)
    ]
    if RUNNER is not None:
        results = RUNNER(nc, in_maps)
    else:
        if not getattr(nc, "_waitfix_done", False):
            _fix_multiwait(nc)
            nc._waitfix_done = True
        res = run_bass_kernel_spmd(nc, in_maps, list(range(N_CORES)))
        results = res.results

    # ---------------- host: final S-update ----------------
    n = N_CORES
    gmask64 = consts["gmask"].astype(np.float64)
    selK64 = consts["selK"].astype(np.float64)
    per = []
    St = None
    for i in range(n):
        r = results[i]
        kg = selK64.T @ (r["gram_raw"].astype(np.float64) * gmask64)  # [8,177]
        M = kg[:, 0:128].reshape(8, K, 8).sum(axis=1)
        Bt = kg[:, 128:176].reshape(8, K, 3).sum(axis=1)
        vsum = kg[:, 176]
        vsq = np.diag(M).copy()
        f_prev = r["fs_out"][:, 0].astype(np.float64)
        St = r["fs_out"][:, 1:4].astype(np.float64)
        per.append(dict(M=M, Bt=Bt, vsum=vsum, vsq=vsq, f_prev=f_prev))
    S_cur = St.T                                   # [3,8] S at final iter start
    A = S_cur.T @ S_cur
    tauD = 1.0 / A.trace()
    snrm = np.sqrt(np.diag(A))

    meanX_all = Xf.mean(axis=2, dtype=np.float64)  # [n,3]
    sumGt = np.zeros((8, 3))
    dtn_sum = np.zeros(8)
    fro_sum = 0.0
    x0s = np.zeros((n, 3))
    f1s = []
    for i in range(n):
        p = per[i]
        sqv = np.sqrt(p["vsq"])
        DtL2 = p["f_prev"] * sqv
        with np.errstate(divide="ignore", invalid="ignore"):
            scl = np.maximum(DtL2 - lam_a * tauD * snrm, 0.0) / DtL2 + EPS
        f1 = p["f_prev"] * scl
        f1s.append(f1)
        fvs = f1 * p["vsum"]
        x0 = meanX_all[i] + (S_cur @ fvs) / P
        x0s[i] = x0
        Mff = np.outer(f1, f1) * p["M"]
        Gt = Mff @ S_cur.T + p["Bt"] * f1[:, None] - np.outer(fvs, x0)
        sumGt += Gt
        dtn_sum += gamma_a * fvs + f1 * sqv
        fro_sum += (f1 ** 2 * p["vsq"]).sum()
    tauS = 1.0 / (fro_sum / n)
    Sg = S_cur - tauS * (sumGt.T / n)              # [3,8]
    dtn = dtn_sum / n
    snrm2 = np.sqrt((Sg ** 2).sum(axis=0))         # [8]
    sclS = np.maximum(snrm2 - lam_a * tauS * dtn, 0.0) / (snrm2 + EPS)
    S_new = Sg * sclS
    nrm3 = np.sqrt((S_new ** 2).sum(axis=0))       # [8]
    S_final = S_new / (nrm3 + EPS)
    f_finals = [f1s[i] * (nrm3 + EPS) for i in range(n)]

    # outputs
    x0_out = x0s.reshape(n, 3, 1).astype(np.float32)
    S_out = S_final.astype(np.float32)
    Dt = np.empty((n, R, P1, P2), dtype=np.float32)
    for i in range(n):
        v = results[i]["V_out"]                    # [128, J]
        vv = v.reshape(K, 8, J).transpose(1, 0, 2).reshape(8, P)
        Dt[i] = (vv * f_finals[i][:, None].astype(np.float32)).reshape(
            R, P1, P2)
    return x0_out, S_out, Dt


# revision 24
# speedup vs baseline: 1.0914x; 1.0914x over previous
"""BeerLaNet proximal-gradient kernel for Trainium2, data-parallel over batch.

Contract: kernel(**inputs) takes FULL inputs (X [8,3,512,512], S [3,8],
gamma [1], lam [1], n_iter) and returns (x0 [8,3,1], S [3,8], Dt [8,8,512,512])
matching reference.py.

Design (per core = one sample):
  Layout A: SBUF [128, 16384] with partition (k*8+r), free j; p = k*16384+j.
  State Dt is factored as Dt[r,p] = f[r] * V[r,p] with V >= 0 stored on-chip
  (dtype BIGDT) and f tracked as an [8,1] per-row scalar.  All row reductions
  (sums / sums of squares) come out of a PE Gram matmul of W=[V;X;1] against
  itself, so no big reduction passes are needed.

  Per iteration:
    D-update:  PSUM = lhsT_V^T @ V + lhsT_X^T @ [X;1]  (PE, fp32r)
               V <- relu(PSUM)                          (ACT)
      where lhsT_V/lhsT_X fold f, tau_D, S^T X, S^T x0 and the L1 threshold
      so that relu(PSUM) == (prox result)/f exactly.
    Gram:      PE transposes of V tiles -> bf16 -> VV^T and XV^T accumulated
               in PSUM; masked+selector-reduced to M [8,8], Bt [8,3], vsum.
    S-update:  tiny [8,x] ops + one 64-float AllReduce across the 8 cores.
  The last iteration's S-update runs on the host (it only needs the tiny
  Gram stats), overlapping the 8MB V writeback with the device tail.

Measured (trn2, 8 cores, n_iter=3): relative L2 error vs the jax reference
  x0 4.3e-05 / S 8.6e-05 / Dt 3.7e-04 (fp32r state rounding dominates; the
  bf16 Gram contributes ~1e-5 thanks to 262k-term cancellation).
"""
import copy
import numpy as np

from concourse import bass, tile
from concourse import mybir
from concourse.bass_utils import run_bass_kernel_spmd

F32 = mybir.dt.float32
F32R = mybir.dt.float32r
BF16 = mybir.dt.bfloat16
AF = mybir.ActivationFunctionType
ALU = mybir.AluOpType

N, C, R, P1, P2 = 8, 3, 8, 512, 512
P = P1 * P2          # 262144
K = 16               # p-chunks per row
J = P // K           # 16384 free columns
W = 512              # D-sweep chunk width
NCH = J // W         # 32 chunks
NT = J // 128        # 128 gram tiles
EPS = 1e-10

BIGDT = F32R         # dtype of V / X / lhsT for the big matmuls
N_CORES = 8

# ---------------------------------------------------------------- wait fix
MAX_WAITS = 1


def _fix_multiwait(nc, max_waits=MAX_WAITS):
    """walrus here rejects >1 sem-wait per instruction; hoist extras onto
    NoOps inserted before the offender on the same (in-order) engine."""
    proto = nc.sync.nop().ins
    for f in nc.m.functions:
        for bb in f.blocks:
            lst = bb.instructions
            if lst and lst[-1] is proto:
                lst.pop()
    uid = 0
    for f in nc.m.functions:
        for bb in f.blocks:
            lst = bb.instructions
            out = []
            for inst in lst:
                si = getattr(inst, "sync_info", None)
                waits = list(si.on_wait) if si is not None else []
                if len(waits) > max_waits:
                    extras, keep = waits[:-max_waits], waits[-max_waits:]
                    for j in range(0, len(extras), max_waits):
                        nop = copy.deepcopy(proto)
                        nop.name = f"I-waitfix-{uid}"
                        uid += 1
                        nop.engine = inst.engine
                        nop.sync_info = mybir.SyncInfo(
                            on_wait=extras[j : j + max_waits], on_update=[]
                        )
                        out.append(nop)
                    inst.sync_info = mybir.SyncInfo(
                        on_wait=keep, on_update=list(si.on_update)
                    )
                out.append(inst)
            lst[:] = out


# ---------------------------------------------------------------- host consts
def host_consts():
    """Constant tensors shared by all cores."""
    c = {}
    c["ident_r"] = np.eye(128, dtype=np.float32)          # BIGDT identity
    c["ident32"] = np.eye(128, dtype=np.float32)
    # sel128[r', k*8+r] = (r == r')  : [8,1] -> [128,1] broadcast
    sel = np.zeros((8, 128), dtype=np.float32)
    for k in range(K):
        for r in range(8):
            sel[r, k * 8 + r] = 1.0
    c["sel128"] = sel
    # selK[k*8+r, r'] = (r == r')   : k-sum selector
    c["selK"] = sel.T.copy()
    # sel48[c', k*3+c] = (c == c')
    s48 = np.zeros((3, 64), dtype=np.float32)
    for k in range(K):
        for cc in range(3):
            s48[cc, k * 3 + cc] = 1.0
    c["sel48"] = s48
    c["ones18"] = np.ones((1, 8), dtype=np.float32)
    c["ones13"] = np.ones((1, 3), dtype=np.float32)
    c["ones31"] = np.ones((3, 1), dtype=np.float32)
    c["ones81"] = np.ones((8, 1), dtype=np.float32)
    c["i8"] = np.eye(8, dtype=np.float32)
    # block-diag masks
    bd = np.zeros((128, 128), dtype=np.float32)
    for k in range(K):
        bd[k * 8 : k * 8 + 8, k * 8 : k * 8 + 8] = 1.0
    c["bd128"] = bd
    b48 = np.zeros((64, 128), dtype=np.float32)
    for k in range(K):
        b48[k * 3 : k * 3 + 3, k * 8 : k * 8 + 8] = 1.0
    c["bd48"] = b48
    # gram mask [128, 177]
    gm = np.zeros((128, 177), dtype=np.float32)
    for k in range(K):
        gm[k * 8 : k * 8 + 8, k * 8 : k * 8 + 8] = 1.0          # VV^T diag blocks
        gm[k * 8 : k * 8 + 8, 128 + k * 3 : 128 + k * 3 + 3] = 1.0  # XV^T
    gm[:, 176] = 1.0                                            # ones column
    c["gmask"] = gm
    return c


def host_iter0(S, gamma, lam, meanX_core):
    """Host-computed stage-A for iteration 0 (f=1, vsum=0, x0=meanX)."""
    S = S.astype(np.float64)
    A = S.T @ S
    tau_D = 1.0 / A.trace()
    snrm = np.sqrt(np.diag(A))
    thr = lam * gamma * tau_D * snrm                     # [8]
    # lhsT_X rows (k,c): -tau_D * S[c,r]  (f=1)
    lx = np.zeros((65, 128), dtype=np.float32)
    for k in range(K):
        for cc in range(3):
            for r in range(8):
                lx[k * 3 + cc, k * 8 + r] = -tau_D * S[cc, r]
    # c-row: tau_D * (S^T x0)[r] - thr[r] with x0 = meanX
    stx0 = S.T @ meanX_core.reshape(3)
    c2 = tau_D * stx0 - thr
    for k in range(K):
        lx[64, k * 8 : k * 8 + 8] = c2
    return lx.astype(np.float32), tau_D, snrm


# ---------------------------------------------------------------- builder
def build_nc(n_iter: int):
    nc = bass.Bass()
    dram = {}

    def din(name, shape, dt=F32):
        dram[name] = nc.dram_tensor(name, shape, dt, kind="ExternalInput")
        return dram[name]

    def dout(name, shape, dt=F32):
        dram[name] = nc.dram_tensor(name, shape, dt, kind="ExternalOutput")
        return dram[name]

    X_in = din("Xin", [65, J], BIGDT)
    XT_in = din("XTin", [128, NT * 49], BF16)
    lx1_in = din("lx1", [65, 128], BIGDT)
    St_in = din("St0", [8, 3])
    meanX_in = din("meanX", [3, 1])
    f0_in = din("f0", [8, 1])
    tauD0_in = din("tauD0", [8, 1])
    snrm0_in = din("snrm0", [8, 1])
    l8_in = din("l8", [8, 1])        # lam
    lg8_in = din("lg8", [8, 1])      # lam*gamma
    g8_in = din("g8", [8, 1])        # gamma

    consts = host_consts()
    cin = {}
    for name, arr in consts.items():
        dt = BIGDT if name == "ident_r" else (F32)
        cin[name] = din(name, list(arr.shape), dt)

    V_out = dout("V_out", [128, J])
    gram_raw_out = dout("gram_raw", [128, 177])
    fs_out = dout("fs_out", [8, 8])

    cc_groups = [list(range(N_CORES))]

    with tile.TileContext(nc) as tc:
        with tc.tile_pool(name="big", bufs=1) as bigpool, \
             tc.tile_pool(name="stage", bufs=4) as stpool, \
             tc.tile_pool(name="small", bufs=2) as sm, \
             tc.tile_pool(name="psweep", bufs=2, space="PSUM") as psweep, \
             tc.tile_pool(name="pgram", bufs=1, space="PSUM") as pgram, \
             tc.tile_pool(name="ptp", bufs=2, space="PSUM") as ptp, \
             tc.tile_pool(name="psmall", bufs=2, space="PSUM") as psm, \
             tc.tile_pool(name="dram", bufs=2, space="DRAM") as dpool:

            # ---------- persistent SBUF state ----------
            V = bigpool.tile([128, J], BIGDT, tag="V", name="V")
            X49 = bigpool.tile([65, J], BIGDT, tag="X49", name="X49")
            XT16 = bigpool.tile([128, NT * 49], BF16, tag="XT16", name="XT16")
            nc.sync.dma_start(out=X49[:], in_=X_in[:])
            nc.sync.dma_start(out=XT16[:], in_=XT_in[:])

            cst = {}
            for name, arr in consts.items():
                dt = BIGDT if name == "ident_r" else F32
                cst[name] = bigpool.tile(list(arr.shape), dt, tag=f"c_{name}", name=f"c_{name}")
                nc.sync.dma_start(out=cst[name][:], in_=cin[name][:])

            lxt1 = bigpool.tile([65, 128], BIGDT, tag="lx1", name="lx1t")
            nc.sync.dma_start(out=lxt1[:], in_=lx1_in[:])
            St = bigpool.tile([8, 3], F32, tag="St", name="Stt")
            nc.sync.dma_start(out=St[:], in_=St_in[:])
            meanX = bigpool.tile([3, 1], F32, tag="meanX", name="meanXt")
            nc.sync.dma_start(out=meanX[:], in_=meanX_in[:])
            smallins = {}
            for name, t_in in (("f0", f0_in), ("tauD0", tauD0_in),
                               ("snrm0", snrm0_in), ("l8", l8_in),
                               ("lg8", lg8_in), ("g8", g8_in)):
                smallins[name] = bigpool.tile([8, 1], F32, tag=f"s_{name}", name=f"s_{name}")
                nc.sync.dma_start(out=smallins[name][:], in_=t_in[:])

            ident_r = cst["ident_r"]
            i8 = cst["i8"]

            # helpers --------------------------------------------------
            def ts(out, in0, s1, op0, s2=None, op1=None):
                if op1 is None:
                    nc.vector.tensor_scalar(out, in0, s1, None, op0)
                else:
                    nc.vector.tensor_scalar(out, in0, s1, s2, op0, op1)

            def tt(out, a, b, op):
                nc.vector.tensor_tensor(out=out, in0=a, in1=b, op=op)

            def small(shape, tag, dt=F32):
                return sm.tile(shape, dt, tag=tag, name=tag)

            def psmall(shape, tag):
                return psm.tile(shape, F32, tag="ps_small", name=tag)

            def pe_bcast(rhs_ap, npart, tag):
                """broadcast a [1, m] row (or [1,1]) to [npart, m] via PE."""
                lhs = cst["ones18"][:, 0:npart] if npart <= 8 else None
                assert lhs is not None
                m = rhs_ap.shape[-1]
                ps = psmall([npart, m], tag)
                nc.tensor.matmul(ps[:], lhs, rhs_ap, start=True, stop=True)
                return ps

            def transpose_small(in_ap, rows, cols, tag):
                """[rows, cols] -> [cols, rows] via PE (fp32)."""
                ps = psm.tile([cols, rows], F32, tag="ps_small", name=tag)
                nc.tensor.transpose(ps[:], in_ap, cst["ident32"][0:rows, 0:rows])
                out = small([cols, rows], tag + "_sb")
                nc.vector.tensor_copy(out[:], ps[:])
                return out

            # ---------- per-iteration python-side state ----------
            f8 = smallins["f0"]          # [8,1] current f
            tauD8 = smallins["tauD0"]    # [8,1]
            snrm8 = smallins["snrm0"]    # [8,1]
            lxt = lxt1                   # current lhsT_X [49,128] BIGDT
            lvt = None                   # current lhsT_V [128,128] BIGDT
            l8 = smallins["l8"]
            lg8 = smallins["lg8"]
            g8 = smallins["g8"]

            for it in range(n_iter):
                last = it == n_iter - 1
                # ================= D-update sweep =================
                gpsA = pgram.tile([128, 128], F32, tag="gramA", name="gramA")
                gpsB = pgram.tile([128, 49], F32, tag="gramB", name="gramB")

                def gram_group(g):
                    tp = ptp.tile([128, 512], BIGDT, tag="tp", name="tp")
                    for u in range(4):
                        t = g * 4 + u
                        nc.tensor.transpose(tp[:, u * 128 : u * 128 + 128],
                                            V[:, t * 128 : t * 128 + 128],
                                            ident_r[:])
                    st = stpool.tile([128, 512], BF16, tag="vt16", name="vt16")
                    nc.vector.tensor_copy(st[:], tp[:].bitcast(F32))
                    for u in range(4):
                        t = g * 4 + u
                        usl = slice(u * 128, u * 128 + 128)
                        nc.tensor.matmul(gpsA[:], st[:, usl], st[:, usl],
                                         start=(t == 0), stop=(t == NT - 1))
                        nc.tensor.matmul(gpsB[:], st[:, usl],
                                         XT16[:, t * 49 : t * 49 + 49],
                                         start=(t == 0), stop=(t == NT - 1))

                # D-update sweep with the gram for chunk ch-1 interleaved so
                # PE fills its ACT-wait stalls with transpose/gram work.
                for ch in range(NCH):
                    sl = slice(ch * W, (ch + 1) * W)
                    ps = psweep.tile([128, W], F32, tag="sweep", name="sweep")
                    if it > 0:
                        nc.tensor.matmul(ps[:], lvt[:], V[:, sl], start=True,
                                         stop=False)
                    nc.tensor.matmul(ps[:], lxt[:], X49[:, sl], start=(it == 0),
                                     stop=True)
                    nc.scalar.activation(V[:, sl], ps[:], AF.Relu)
                    if last:
                        # stream V out as soon as each chunk is final
                        nc.sync.dma_start(out=V_out[:, sl],
                                          in_=V[:, sl].bitcast(F32))
                    if ch >= 1:
                        gram_group(ch - 1)
                gram_group(NCH - 1)

                if last:
                    go = small([128, 177], "gramraw")
                    nc.vector.tensor_copy(go[:, 0:128], gpsA[:])
                    nc.vector.tensor_copy(go[:, 128:177], gpsB[:])
                    nc.sync.dma_start(out=gram_raw_out[:], in_=go[:])
                    fsb = small([8, 8], "fsb")
                    nc.vector.memset(fsb[:], 0.0)
                    nc.vector.tensor_copy(fsb[:, 0:1], f8[:])
                    nc.vector.tensor_copy(fsb[:, 1:4], St[:])
                    nc.sync.dma_start(out=fs_out[:], in_=fsb[:])
                    break

                # ---------- Gram reduce: M, Bt, vsum, vsq ----------
                gsb = small([128, 177], "gsb")
                tt(gsb[:, 0:128], gpsA[:], cst["gmask"][:, 0:128], ALU.mult)
                tt(gsb[:, 128:177], gpsB[:], cst["gmask"][:, 128:177], ALU.mult)
                kgp = psm.tile([8, 177], F32, tag="ps_small", name="kgp")
                nc.tensor.matmul(kgp[:], cst["selK"][:], gsb[:], start=True,
                                 stop=True)
                KG = small([8, 177], "KG")
                nc.vector.tensor_copy(KG[:], kgp[:])
                M = small([8, 8], "M")
                nc.vector.tensor_reduce(
                    M[:], KG[:, 0:128].rearrange("p (k r) -> p r k", k=K, r=8),
                    mybir.AxisListType.X, ALU.add)
                Bt = small([8, 3], "Bt")
                nc.vector.tensor_reduce(
                    Bt[:], KG[:, 128:176].rearrange("p (k c) -> p c k", k=K, c=3),
                    mybir.AxisListType.X, ALU.add)
                vsum = small([8, 1], "vsum")
                nc.vector.tensor_copy(vsum[:], KG[:, 176:177])
                vsq = small([8, 1], "vsq")
                scr8 = small([8, 8], "scr8")
                tt(scr8[:], M[:], i8[:], ALU.mult)
                nc.vector.tensor_reduce(vsq[:], scr8[:], mybir.AxisListType.X,
                                        ALU.add)

                # ---------- local D-prox scalars ----------
                sqv = small([8, 1], "sqv")
                nc.scalar.activation(sqv[:], vsq[:], AF.Sqrt)
                DtL2 = small([8, 1], "DtL2")
                tt(DtL2[:], f8[:], sqv[:], ALU.mult)
                thrB = small([8, 1], "thrB")
                ts(thrB[:], snrm8[:], l8[:], ALU.mult)
                ts(thrB[:], thrB[:], tauD8[:], ALU.mult)
                t3 = small([8, 1], "t3")
                tt(t3[:], DtL2[:], thrB[:], ALU.subtract)
                ts(t3[:], t3[:], 0.0, ALU.max)
                recL2 = small([8, 1], "recL2")
                nc.vector.reciprocal(recL2[:], DtL2[:])
                scl = small([8, 1], "scl")
                tt(scl[:], t3[:], recL2[:], ALU.mult)
                ts(scl[:], scl[:], EPS, ALU.add)
                f1 = small([8, 1], "f1")
                tt(f1[:], f8[:], scl[:], ALU.mult)

                # ---------- pre-reduce: G^T etc ----------
                fvs = small([8, 1], "fvs")
                tt(fvs[:], f1[:], vsum[:], ALU.mult)
                x0p = psmall([3, 1], "x0p")
                nc.tensor.matmul(x0p[:], St[:], fvs[:], start=True, stop=True)
                x0 = small([3, 1], "x0")
                nc.vector.tensor_scalar(x0[:], x0p[:], 1.0 / P, None, ALU.mult)
                tt(x0[:], x0[:], meanX[:], ALU.add)

                f1row = transpose_small(f1[:], 8, 1, "f1row")      # [1,8]
                fbc = pe_bcast(f1row[:], 8, "fbc")                 # [8,8] psum
                Mf = small([8, 8], "Mf")
                ts(Mf[:], M[:], f1[:], ALU.mult)
                tt(Mf[:], Mf[:], fbc[:], ALU.mult)                 # f_i f_j M
                Btf = small([8, 3], "Btf")
                ts(Btf[:], Bt[:], f1[:], ALU.mult)

                gt_ps = psmall([8, 3], "gt")
                nc.tensor.matmul(gt_ps[:], Mf[:], St[:], start=True, stop=False)
                nc.tensor.matmul(gt_ps[:], i8[:], Btf[:], start=False, stop=False)
                nfvs = small([8, 1], "nfvs")
                ts(nfvs[:], fvs[:], -1.0, ALU.mult)
                nfvs_row = transpose_small(nfvs[:], 8, 1, "nfvsrow")  # [1,8]
                x0row = transpose_small(x0[:], 3, 1, "x0row")         # [1,3]
                nc.tensor.matmul(gt_ps[:], nfvs_row[:], x0row[:], start=False,
                                 stop=True)
                Gt = small([8, 3], "Gt")
                nc.vector.tensor_copy(Gt[:], gt_ps[:])

                dtn = small([8, 1], "dtn")
                tt(dtn[:], g8[:], fvs[:], ALU.mult)
                tb = small([8, 1], "tb_")
                tt(tb[:], f1[:], sqv[:], ALU.mult)
                tt(dtn[:], dtn[:], tb[:], ALU.add)
                fro = small([8, 1], "fro")
                tt(fro[:], f1[:], f1[:], ALU.mult)
                tt(fro[:], fro[:], vsq[:], ALU.mult)

                alr = small([8, 8], "alr")
                nc.vector.memset(alr[:], 0.0)
                nc.vector.tensor_copy(alr[:, 0:3], Gt[:])
                nc.vector.tensor_copy(alr[:, 3:4], dtn[:])
                nc.vector.tensor_copy(alr[:, 4:5], fro[:])

                cc_in = dpool.tile([8, 8], F32, name="cc_in")
                cc_out = dpool.tile([8, 8], F32, addr_space="Shared", name="cc_out")
                nc.sync.dma_start(out=cc_in[:], in_=alr[:])
                nc.gpsimd.collective_compute(
                    "AllReduce", ALU.add, replica_groups=cc_groups,
                    ins=[cc_in.opt()], outs=[cc_out.opt()])
                Rr = small([8, 8], "Rr")
                nc.sync.dma_start(out=Rr[:], in_=cc_out[:])

                # ---------- stage B: S update ----------
                frosum_ps = psmall([1, 1], "frosum")
                nc.tensor.matmul(frosum_ps[:], cst["ones81"][:], Rr[:, 4:5],
                                 start=True, stop=True)
                tauS = small([1, 1], "tauS")
                nc.vector.reciprocal(tauS[:], frosum_ps[:])
                ts(tauS[:], tauS[:], float(N_CORES), ALU.mult)
                tauS8 = psm.tile([8, 1], F32, tag="ps_small", name="tauS8")
                nc.tensor.matmul(tauS8[:], cst["ones18"][:], tauS[:],
                                 start=True, stop=True)
                tauS8s = small([8, 1], "tauS8s")
                nc.vector.tensor_copy(tauS8s[:], tauS8[:])

                Sg = small([8, 3], "Sg")
                ts(Sg[:], Rr[:, 0:3], tauS8s[:], ALU.mult, 1.0 / N_CORES,
                   ALU.mult)
                tt(Sg[:], St[:], Sg[:], ALU.subtract)
                scrg = small([8, 3], "scrg")
                sg2 = small([8, 1], "sg2")
                tt(scrg[:], Sg[:], Sg[:], ALU.mult)
                nc.vector.tensor_reduce(sg2[:], scrg[:], mybir.AxisListType.X,
                                        ALU.add)
                snrm2 = small([8, 1], "snrm2")
                nc.scalar.activation(snrm2[:], sg2[:], AF.Sqrt)
                dtnm = small([8, 1], "dtnm")
                ts(dtnm[:], Rr[:, 3:4], 1.0 / N_CORES, ALU.mult)
                t1 = small([8, 1], "sb_t1")
                tt(t1[:], dtnm[:], tauS8s[:], ALU.mult)
                tt(t1[:], t1[:], l8[:], ALU.mult)
                t2 = small([8, 1], "sb_t2")
                tt(t2[:], snrm2[:], t1[:], ALU.subtract)
                ts(t2[:], t2[:], 0.0, ALU.max)
                t5 = small([8, 1], "sb_t5")
                ts(t5[:], snrm2[:], EPS, ALU.add)
                rec5 = small([8, 1], "sb_rec5")
                nc.vector.reciprocal(rec5[:], t5[:])
                sclS = small([8, 1], "sclS")
                tt(sclS[:], t2[:], rec5[:], ALU.mult)
                Sn = small([8, 3], "Sn")
                ts(Sn[:], Sg[:], sclS[:], ALU.mult)
                ns_scr = small([8, 3], "ns_scr")
                ns2 = small([8, 1], "ns2")
                tt(ns_scr[:], Sn[:], Sn[:], ALU.mult)
                nc.vector.tensor_reduce(ns2[:], ns_scr[:], mybir.AxisListType.X,
                                        ALU.add)
                nrm3 = small([8, 1], "nrm3")
                nc.scalar.activation(nrm3[:], ns2[:], AF.Sqrt)
                nrm3e = small([8, 1], "nrm3e")
                ts(nrm3e[:], nrm3[:], EPS, ALU.add)
                recn3 = small([8, 1], "recn3")
                nc.vector.reciprocal(recn3[:], nrm3e[:])
                St_new = small([8, 3], "St_new")
                ts(St_new[:], Sn[:], recn3[:], ALU.mult)
                f_new = small([8, 1], "f_new")
                tt(f_new[:], f1[:], nrm3e[:], ALU.mult)

                # ---------- stage A for next iteration ----------
                Ssb_ps = psm.tile([3, 8], F32, tag="ps_small", name="Ssb_ps")
                nc.tensor.transpose(Ssb_ps[:], St_new[:],
                                    cst["ident32"][0:8, 0:8])
                Ssb = small([3, 8], "Ssb")
                nc.vector.tensor_copy(Ssb[:], Ssb_ps[:])

                A_ps = psmall([8, 8], "A_ps")
                nc.tensor.matmul(A_ps[:], Ssb[:], Ssb[:], start=True, stop=True)
                Amat = small([8, 8], "Amat")
                nc.vector.tensor_copy(Amat[:], A_ps[:])
                Ssq = small([3, 8], "Ssq")
                nc.scalar.activation(Ssq[:], Ssb[:], AF.Square)
                n2_ps = psmall([1, 8], "n2_ps")
                nc.tensor.matmul(n2_ps[:], cst["ones31"][:], Ssq[:], start=True,
                                 stop=True)
                n2row = small([1, 8], "n2row")
                nc.vector.tensor_copy(n2row[:], n2_ps[:])
                tot = small([1, 1], "tot")
                nc.vector.tensor_reduce(tot[:], n2row[:], mybir.AxisListType.X,
                                        ALU.add)
                tauD = small([1, 1], "tauD")
                nc.vector.reciprocal(tauD[:], tot[:])
                snrmrow = small([1, 8], "snrmrow")
                nc.scalar.activation(snrmrow[:], n2row[:], AF.Sqrt)
                snrm8_n = transpose_small(snrmrow[:], 1, 8, "snrm8n")  # [8,1]
                tauD8n_ps = pe_bcast(tauD[:], 8, "tauD8n")
                tauD8n = small([8, 1], "tauD8nsb")
                nc.vector.tensor_copy(tauD8n[:], tauD8n_ps[:])
                tauD3_ps = pe_bcast(tauD[:], 3, "tauD3")
                tauD3 = small([3, 1], "tauD3sb")
                nc.vector.tensor_copy(tauD3[:], tauD3_ps[:])

                recf = small([8, 1], "recf")
                nc.vector.reciprocal(recf[:], f_new[:])
                recfrow = transpose_small(recf[:], 8, 1, "recfrow")   # [1,8]

                # B = I - tauD * f_i A_ij / f_j
                bt1 = small([8, 8], "bt1")
                ts(bt1[:], Amat[:], f_new[:], ALU.mult)
                recbc = pe_bcast(recfrow[:], 8, "recbc")              # [8,8]
                tt(bt1[:], bt1[:], recbc[:], ALU.mult)
                ts(bt1[:], bt1[:], tauD8n[:], ALU.mult)
                Bm = small([8, 8], "Bm")
                tt(Bm[:], i8[:], bt1[:], ALU.subtract)
                bcB_ps = psm.tile([128, 8], F32, tag="ps_small", name="bcB_ps")
                nc.tensor.matmul(bcB_ps[:], cst["sel128"][:], Bm[:], start=True,
                                 stop=True)
                bcB = small([128, 8], "bcBsb")
                nc.vector.tensor_copy(bcB[:], bcB_ps[:])
                lvt_f = stpool.tile([128, 128], F32, tag="lvtf", name="lvtf")
                tt(lvt_f[:].rearrange("p (k r) -> p k r", k=K, r=8),
                   bcB[:].unsqueeze(1).broadcast_to([128, K, 8]),
                   cst["bd128"][:].rearrange("p (k r) -> p k r", k=K, r=8),
                   ALU.mult)
                lvt_new = stpool.tile([128, 128], BIGDT, tag="lvt", name="lvt")
                ts(lvt_new[:], lvt_f[:], 1.0, ALU.mult)

                # lhsT_X rows: -tauD * S[c,r] / f[r]
                xb1 = small([3, 8], "xb1")
                rec3bc = pe_bcast(recfrow[:], 3, "rec3bc")            # [3,8]
                tt(xb1[:], Ssb[:], rec3bc[:], ALU.mult)
                ts(xb1[:], xb1[:], tauD3[:], ALU.mult)
                ts(xb1[:], xb1[:], -1.0, ALU.mult)
                bc48_ps = psm.tile([64, 8], F32, tag="ps_small", name="bc48_ps")
                nc.tensor.matmul(bc48_ps[:], cst["sel48"][:], xb1[:],
                                 start=True, stop=True)
                bc48 = small([64, 8], "bc48sb")
                nc.vector.tensor_copy(bc48[:], bc48_ps[:])
                lxt_f = stpool.tile([65, 128], F32, tag="lxtf", name="lxtf")
                tt(lxt_f[0:64, :].rearrange("p (k r) -> p k r", k=K, r=8),
                   bc48[:].unsqueeze(1).broadcast_to([64, K, 8]),
                   cst["bd48"][:].rearrange("p (k r) -> p k r", k=K, r=8),
                   ALU.mult)

                # c-row: (tauD * (S^T x0') - thr) / f ; x0' uses f_new,vsum
                fvs2 = small([8, 1], "fvs2")
                tt(fvs2[:], f_new[:], vsum[:], ALU.mult)
                x0n_ps = psmall([3, 1], "x0n_ps")
                nc.tensor.matmul(x0n_ps[:], St_new[:], fvs2[:], start=True,
                                 stop=True)
                x0n = small([3, 1], "x0n")
                nc.vector.tensor_scalar(x0n[:], x0n_ps[:], 1.0 / P, None,
                                        ALU.mult)
                tt(x0n[:], x0n[:], meanX[:], ALU.add)
                stx0_ps = psmall([8, 1], "stx0")
                nc.tensor.matmul(stx0_ps[:], Ssb[:], x0n[:], start=True,
                                 stop=True)
                c2 = small([8, 1], "c2")
                nc.vector.tensor_scalar(c2[:], stx0_ps[:], tauD8n[:], None,
                                        ALU.mult)
                thrN = small([8, 1], "thrN")
                tt(thrN[:], snrm8_n[:], lg8[:], ALU.mult)
                tt(thrN[:], thrN[:], tauD8n[:], ALU.mult)
                tt(c2[:], c2[:], thrN[:], ALU.subtract)
                tt(c2[:], c2[:], recf[:], ALU.mult)
                c2row = transpose_small(c2[:], 8, 1, "c2row")         # [1,8]
                nc.vector.tensor_copy(
                    lxt_f[64:65, :].rearrange("p (k r) -> p k r", k=K, r=8),
                    c2row[:].unsqueeze(1).broadcast_to([1, K, 8]))
                lxt_new = stpool.tile([65, 128], BIGDT, tag="lxt", name="lxt")
                ts(lxt_new[:], lxt_f[:], 1.0, ALU.mult)

                # rotate python-side state
                f8, tauD8, snrm8 = f_new, tauD8n, snrm8_n
                lvt, lxt, St = lvt_new, lxt_new, St_new

    return nc


# ---------------------------------------------------------------- host side
def _prep_core_inputs(Xi, S, gamma, lam, consts):
    """Xi: [3, P] float32 for this core."""
    out = {}
    Xk = Xi.reshape(3, K, J)                       # [c,k,j]
    x49 = np.zeros((65, J), dtype=np.float32)
    for k in range(K):
        x49[k * 3 : k * 3 + 3, :] = Xk[:, k, :]
    x49[64, :] = 1.0
    out["Xin"] = x49
    # XT16[q, t*49 + k*3+c] = X[c, k*16384 + t*128 + q]; col 48 of block = 1
    Xt = Xi.reshape(3, K, NT, 128)                 # [c,k,t,q]
    xt = np.ones((128, NT, 49), dtype=np.float32)
    # -> [q, t, k*3+c]
    xt[:, :, 0:48] = np.transpose(Xt, (3, 2, 1, 0)).reshape(128, NT, 48)
    out["XTin"] = xt.reshape(128, NT * 49).astype(mybir.dt.np(BF16))
    out["meanX"] = Xi.mean(axis=1, dtype=np.float64).astype(np.float32).reshape(3, 1)
    lx1, tauD, snrm = host_iter0(S, float(gamma), float(lam), out["meanX"])
    out["lx1"] = lx1
    out["St0"] = S.T.astype(np.float32).copy()
    out["f0"] = np.ones((8, 1), np.float32)
    out["tauD0"] = np.full((8, 1), tauD, np.float32)
    out["snrm0"] = snrm.reshape(8, 1).astype(np.float32)
    out["l8"] = np.full((8, 1), float(lam), np.float32)
    out["lg8"] = np.full((8, 1), float(lam) * float(gamma), np.float32)
    out["g8"] = np.full((8, 1), float(gamma), np.float32)
    for name, arr in consts.items():
        out[name] = arr
    return out


RUNNER = None  # test hook: (nc, in_maps) -> results
_NC_CACHE = {}


def kernel(X, S, gamma, lam, n_iter):
    n_iter = int(n_iter)
    X = np.asarray(X, dtype=np.float32)
    S = np.asarray(S, dtype=np.float32)
    gamma_a = abs(float(np.asarray(gamma).reshape(-1)[0]))
    lam_a = abs(float(np.asarray(lam).reshape(-1)[0]))

    if n_iter not in _NC_CACHE:
        _NC_CACHE[n_iter] = build_nc(n_iter)
    nc = _NC_CACHE[n_iter]

    consts = host_consts()
    Xf = X.reshape(N, C, P)
    in_maps = [
        _prep_core_inputs(Xf[i], S, gamma_a, lam_a, consts) for i in range(N)
    ]
    if RUNNER is not None:
        results = RUNNER(nc, in_maps)
    else:
        if not getattr(nc, "_waitfix_done", False):
            _fix_multiwait(nc)
            nc._waitfix_done = True
        res = run_bass_kernel_spmd(nc, in_maps, list(range(N_CORES)))
        results = res.results

    # ---------------- host: final S-update ----------------
    n = N_CORES
    gmask64 = consts["gmask"].astype(np.float64)
    selK64 = consts["selK"].astype(np.float64)
    per = []
    St = None
    for i in range(n):
        r = results[i]
        kg = selK64.T @ (r["gram_raw"].astype(np.float64) * gmask64)  # [8,177]
        M = kg[:, 0:128].reshape(8, K, 8).sum(axis=1)
        Bt = kg[:, 128:176].reshape(8, K, 3).sum(axis=1)
        vsum = kg[:, 176]
        vsq = np.diag(M).copy()
        f_prev = r["fs_out"][:, 0].astype(np.float64)
        St = r["fs_out"][:, 1:4].astype(np.float64)
        per.append(dict(M=M, Bt=Bt, vsum=vsum, vsq=vsq, f_prev=f_prev))
    S_cur = St.T                                   # [3,8] S at final iter start
    A = S_cur.T @ S_cur
    tauD = 1.0 / A.trace()
    snrm = np.sqrt(np.diag(A))

    meanX_all = Xf.mean(axis=2, dtype=np.float64)  # [n,3]
    sumGt = np.zeros((8, 3))
    dtn_sum = np.zeros(8)
    fro_sum = 0.0
    x0s = np.zeros((n, 3))
    f1s = []
    for i in range(n):
        p = per[i]
        sqv = np.sqrt(p["vsq"])
        DtL2 = p["f_prev"] * sqv
        with np.errstate(divide="ignore", invalid="ignore"):
            scl = np.maximum(DtL2 - lam_a * tauD * snrm, 0.0) / DtL2 + EPS
        f1 = p["f_prev"] * scl
        f1s.append(f1)
        fvs = f1 * p["vsum"]
        x0 = meanX_all[i] + (S_cur @ fvs) / P
        x0s[i] = x0
        Mff = np.outer(f1, f1) * p["M"]
        Gt = Mff @ S_cur.T + p["Bt"] * f1[:, None] - np.outer(fvs, x0)
        sumGt += Gt
        dtn_sum += gamma_a * fvs + f1 * sqv
        fro_sum += (f1 ** 2 * p["vsq"]).sum()
    tauS = 1.0 / (fro_sum / n)
    Sg = S_cur - tauS * (sumGt.T / n)              # [3,8]
    dtn = dtn_sum / n
    snrm2 = np.sqrt((Sg ** 2).sum(axis=0))         # [8]
    sclS = np.maximum(snrm2 - lam_a * tauS * dtn, 0.0) / (snrm2 + EPS)
    S_new = Sg * sclS
    nrm3 = np.sqrt((S_new ** 2).sum(axis=0))       # [8]
    S_final = S_new / (nrm3 + EPS)
    f_finals = [f1s[i] * (nrm3 + EPS) for i in range(n)]

    # outputs
    x0_out = x0s.reshape(n, 3, 1).astype(np.float32)
    S_out = S_final.astype(np.float32)
    Dt = np.empty((n, R, P1, P2), dtype=np.float32)
    for i in range(n):
        v = results[i]["V_out"]                    # [128, J]
        vv = v.reshape(K, 8, J).transpose(1, 0, 2).reshape(8, P)
        Dt[i] = (vv * f_finals[i][:, None].astype(np.float32)).reshape(
            R, P1, P2)
    return x0_out, S_out, Dt
